# revision 2
# baseline (speedup 1.0000x reference)
"""Trainium2 Bass kernel for per-batch spatial self-attention — fp8 DoubleRow.

Per-core computation (one batch image per NeuronCore, 8 cores):
  x:(256, 4096) -> q/k = W x + b (channels-major, fp8e4m3 storage)
                   v = x^T W^T + b (pixels-major, fp8e4m3 storage)
  St[m,n] = sum_c K[c,m] Q[c,n]   via fp8 DoubleRow matmul (256-deep contraction
                                  in one instruction: channel pairs packed as
                                  [p, 2, .] access patterns)
  Pt = exp(St/16) -> fp8          one Act instruction per 2-bank PSUM tile
  OT[o,n] = sum_m V[m,o] Pt[m,n]  fp8 DoubleRow, key pairs
  rowsum via fp8 DoubleRow ones-matmul accumulated across key pairs in PSUM
  out = OT * (1/rowsum)           deferred ~2 tiles so the reciprocal chain
                                  never stalls the PE queue

Projections stay in float32r; q/k stored bf16 (fp8 q/k fails the 2e-2 gate:
query-side quantization error is correlated across keys), pt/v in fp8e4m3.
"""

import sys

sys.path.insert(0, "/opt/trn_rl_repo")

import numpy as np
import concourse.bacc as bacc
import concourse.bass as bass
import concourse.mybir as mybir
import concourse.tile as tile
from concourse.bass_utils import run_bass_kernel_spmd

F32 = mybir.dt.float32
F32R = mybir.dt.float32r
FP8 = mybir.dt.float8e4
BF16 = mybir.dt.bfloat16
AF = mybir.ActivationFunctionType
DR = mybir.MatmulPerfMode.DoubleRow

B = 8
C = 256  # channels
NPIX = 4096  # 64*64
NT = NPIX // 512  # 8 query tiles of 512
NJ = NPIX // 256  # 16 key-chunk pairs (2x128) per query tile
SCALE = 1.0 / 16.0  # 1/sqrt(C)
LAG = 3  # OT stream lags the score stream by this many (nt,j) steps
EPI_DELAY = 2  # epilogue part B (rb matmul + muls) deferred this many steps

_CACHE = {}


def _build():
    nc = bacc.Bacc("TRN2", num_swdge_queues=4)
    x_d = nc.declare_dram_parameter("x", [C, NPIX], F32, isOutput=False)
    wq_d = nc.declare_dram_parameter("wq_t", [C, C], F32, isOutput=False)
    wk_d = nc.declare_dram_parameter("wk_t", [C, C], F32, isOutput=False)
    wv_d = nc.declare_dram_parameter("wv_t", [C, C], F32, isOutput=False)
    bq_d = nc.declare_dram_parameter("bq", [C, 1], F32, isOutput=False)
    bk_d = nc.declare_dram_parameter("bk", [C, 1], F32, isOutput=False)  # unused
    bv_d = nc.declare_dram_parameter("bv", [1, C], F32, isOutput=False)
    out_d = nc.declare_dram_parameter("out", [C, NPIX], F32, isOutput=True)

    with tile.TileContext(nc) as tc:
        with (
            tc.tile_pool(name="big", bufs=1) as big,
            tc.tile_pool(name="small", bufs=2) as small,
            tc.tile_pool(name="ptp", bufs=LAG + 3) as ptp,
            tc.tile_pool(name="up", bufs=2) as up,
            tc.tile_pool(name="osbp", bufs=2) as osbp,
            tc.tile_pool(name="stp", bufs=2, space="PSUM") as stp,
            tc.tile_pool(name="psO", bufs=1, space="PSUM") as psO,
            tc.tile_pool(name="psR", bufs=1, space="PSUM") as psR,
            tc.tile_pool(name="psQ", bufs=1, space="PSUM") as psQ,
        ):
            # ---- input DMAs (gpsimd casts f32 -> f32r), need-ordered ----
            w_r = {}
            for nm, wd in (("q", wq_d), ("k", wk_d), ("v", wv_d)):
                w_r[nm] = [
                    big.tile([128, C], F32R, name=f"w{nm}_r{i}") for i in range(2)
                ]
            x_r = [big.tile([128, NPIX], F32R, name=f"x_r{i}") for i in range(2)]

            for i in range(2):
                nc.gpsimd.dma_start(
                    out=w_r["k"][i], in_=wk_d[i * 128 : (i + 1) * 128, :]
                )
            for i in range(2):
                nc.gpsimd.dma_start(
                    out=x_r[i][:, 0:512], in_=x_d[i * 128 : (i + 1) * 128, 0:512]
                )
            for i in range(2):
                nc.gpsimd.dma_start(
                    out=w_r["q"][i], in_=wq_d[i * 128 : (i + 1) * 128, :]
                )
            for i in range(2):
                nc.gpsimd.dma_start(
                    out=x_r[i][:, 512:1024],
                    in_=x_d[i * 128 : (i + 1) * 128, 512:1024],
                )
            for i in range(2):
                nc.gpsimd.dma_start(
                    out=w_r["v"][i], in_=wv_d[i * 128 : (i + 1) * 128, :]
                )
            for s in range(2, NT):
                lo, hi = s * 512, (s + 1) * 512
                for i in range(2):
                    nc.gpsimd.dma_start(
                        out=x_r[i][:, lo:hi], in_=x_d[i * 128 : (i + 1) * 128, lo:hi]
                    )

            bq_sb = [big.tile([128, 1], F32, name=f"bq_sb{i}") for i in range(2)]
            for i in range(2):
                nc.sync.dma_start(out=bq_sb[i], in_=bq_d[i * 128 : (i + 1) * 128, :])

            # bv broadcast twice along free: [128, 2, 256]
            bv_bc2 = big.tile([128, 2, C], F32, name="bv_bc2")
            bv_bcast_ap = bass.AP(
                tensor=bv_d.ap().tensor,
                offset=0,
                ap=[[0, 128], [0, 2], [1, C]],
            )
            nc.sync.dma_start(out=bv_bc2, in_=bv_bcast_ap)

            # constants
            ones_f2 = big.tile([128, 2, 32], F32, name="ones_f2")
            nc.vector.memset(ones_f2, 1.0)
            ones8 = big.tile([128, 2, 32], FP8, name="ones8")
            nc.vector.tensor_copy(ones8, ones_f2)
            ones_rf = big.tile([1, 128], F32, name="ones_rf")
            nc.vector.memset(ones_rf, 1.0)
            ones_row = big.tile([1, 128], F32R, name="ones_row")
            nc.vector.tensor_copy(ones_row, ones_rf)

            # q/k in bf16 (query-side fp8 error is key-correlated and
            # dominates the budget); pt/v stay fp8 for DoubleRow AV
            q8 = big.tile([128, 2, NPIX], BF16, name="q8")
            k8 = big.tile([128, 2, NPIX], BF16, name="k8")
            v8 = big.tile([128, NPIX // 128, C], FP8, name="v8")

            # ---- warm up the PE p-state while the first DMAs land ----
            warm_f = small.tile([128, 256], F32, name="warm_f", tag="warm_f")
            nc.vector.memset(warm_f, 1.0)
            warm_r = small.tile([128, 256], F32R, name="warm_r", tag="warm_r")
            nc.vector.tensor_copy(warm_r, warm_f)
            warm_c = small.tile([128, 1], F32R, name="warm_c", tag="warm_c")
            nc.vector.tensor_copy(warm_c, warm_f[:, 0:1])
            warm_ps = stp.tile([128, 2, 512], F32, name="warm_ps", tag="st")
            for _ in range(24):
                nc.tensor.matmul(
                    warm_ps[0:1, 0, 0:256],
                    warm_c,
                    warm_r,
                    start=True,
                    stop=True,
                    skip_group_check=True,
                )

            # ---- projection emitters (f32r matmuls, DVE writes fp8) ----
            def emit_k_slice(s):
                t = stp.tile([128, 2, 512], F32, name="kproj", tag="st")
                for o in range(2):
                    for i in range(2):
                        nc.tensor.matmul(
                            t[:, o, :],
                            w_r["k"][i][:, o * 128 : (o + 1) * 128],
                            x_r[i][:, s * 512 : (s + 1) * 512],
                            start=(i == 0),
                            stop=(i == 1),
                        )
                # k bias is softmax-invariant; skip it
                nc.vector.tensor_copy(k8[:, :, s * 512 : (s + 1) * 512], t)

            def emit_q_slice(s):
                t = stp.tile([128, 2, 512], F32, name="qproj", tag="st")
                for o in range(2):
                    for i in range(2):
                        nc.tensor.matmul(
                            t[:, o, :],
                            w_r["q"][i][:, o * 128 : (o + 1) * 128],
                            x_r[i][:, s * 512 : (s + 1) * 512],
                            start=(i == 0),
                            stop=(i == 1),
                        )
                for o in range(2):
                    nc.vector.tensor_scalar_add(
                        q8[:, o, s * 512 : (s + 1) * 512], t[:, o, :], bq_sb[o]
                    )

            def emit_v_pair(j):
                t = stp.tile([128, 2, 512], F32, name="vproj", tag="st")
                for h in range(2):
                    m = 2 * j + h
                    for i in range(2):
                        nc.tensor.matmul(
                            t[:, h, 0:C],
                            x_r[i][:, m * 128 : (m + 1) * 128],
                            w_r["v"][i],
                            start=(i == 0),
                            stop=(i == 1),
                        )
                nc.vector.tensor_add(
                    v8[:, 2 * j : 2 * j + 2, :], t[:, :, 0:C], bv_bc2
                )

            # ---- main pipeline over (nt, j) steps ----
            pts = {}
            cur = {}
            pending_b = []  # (due_step, nt, u0, u1, rinv_r)

            def emit_ot(nt, j):
                pt_t = pts.pop((nt, j))
                if j == 0:
                    ot0 = psO.tile([128, 512], F32, name="ot0", tag="ot0")
                    ot1 = psO.tile([128, 512], F32, name="ot1", tag="ot1")
                    rs = psR.tile([32, 512], F32, name="rs", tag="rs")
                    cur[nt] = (ot0, ot1, rs)
                ot0, ot1, rs = cur[nt]
                st_, sp_ = (j == 0), (j == NJ - 1)
                nc.tensor.matmul(
                    ot0, v8[:, 2 * j : 2 * j + 2, 0:128], pt_t,
                    start=st_, stop=sp_, perf_mode=DR,
                )
                nc.tensor.matmul(
                    ot1, v8[:, 2 * j : 2 * j + 2, 128:256], pt_t,
                    start=st_, stop=sp_, perf_mode=DR,
                )
                nc.tensor.matmul(
                    rs, ones8, pt_t, start=st_, stop=sp_, perf_mode=DR,
                )

            def emit_epilogue_a(step, nt):
                ot0, ot1, rs = cur.pop(nt)
                u0 = up.tile([128, 512], F32, name="u0", tag="u0")
                nc.vector.tensor_copy(u0, ot0)
                u1 = up.tile([128, 512], F32, name="u1", tag="u1")
                nc.vector.tensor_copy(u1, ot1)
                rinv_f = small.tile([1, 512], F32, name="rinv_f", tag="rinv_f")
                nc.vector.reciprocal_approx_fast(rinv_f, rs[0:1, :])
                rinv_r = small.tile([1, 512], F32R, name="rinv_r", tag="rinv_r")
                nc.vector.tensor_copy(rinv_r, rinv_f)
                pending_b.append((step + EPI_DELAY, nt, u0, u1, rinv_r))

            def emit_epilogue_b(nt, u0, u1, rinv_r):
                rb = psQ.tile([128, 512], F32, name="rb", tag="rb")
                nc.tensor.matmul(rb, ones_row, rinv_r, start=True, stop=True)
                for oc, u in ((0, u0), (1, u1)):
                    osb = osbp.tile([128, 512], F32, name="osb", tag=f"osb{oc}")
                    nc.vector.tensor_mul(osb, u, rb)
                    nc.sync.dma_start(
                        out=out_d[
                            oc * 128 : (oc + 1) * 128, nt * 512 : (nt + 1) * 512
                        ],
                        in_=osb,
                    )

            k_emitted = 0
            q_emitted = 0
            TOTAL = NT * NJ
            for step in range(TOTAL + LAG):
                # flush due epilogue-B work first
                while pending_b and pending_b[0][0] <= step:
                    _, nt_, u0_, u1_, rv_ = pending_b.pop(0)
                    emit_epilogue_b(nt_, u0_, u1_, rv_)

                if step < TOTAL:
                    nt, j = divmod(step, NJ)
                    if nt == 0:
                        need_k = min(NT, (2 * j + 1) // 4 + 1)
                        while k_emitted < need_k:
                            emit_k_slice(k_emitted)
                            k_emitted += 1
                        if q_emitted == 0:
                            emit_q_slice(0)
                            q_emitted = 1
                        emit_v_pair(j)
                    if j == 8 and nt < NT - 1:
                        emit_q_slice(nt + 1)
                        q_emitted += 1
                    # scores + exp (bf16: contraction over c-halves)
                    st_t = stp.tile([128, 2, 512], F32, name="st_t", tag="st")
                    for h in range(2):
                        m = 2 * j + h
                        for i in range(2):
                            nc.tensor.matmul(
                                st_t[:, h, :],
                                k8[:, i, m * 128 : (m + 1) * 128],
                                q8[:, i, nt * 512 : (nt + 1) * 512],
                                start=(i == 0),
                                stop=(i == 1),
                            )
                    pt_t = ptp.tile([128, 2, 512], FP8, name="pt_t", tag="pt")
                    nc.scalar.activation(pt_t, st_t, AF.Exp, scale=SCALE)
                    pts[(nt, j)] = pt_t

                if step >= LAG:
                    nt2, j2 = divmod(step - LAG, NJ)
                    emit_ot(nt2, j2)
                    if j2 == NJ - 1:
                        emit_epilogue_a(step, nt2)

            # final epilogue-B flush
            while pending_b:
                _, nt_, u0_, u1_, rv_ = pending_b.pop(0)
                emit_epilogue_b(nt_, u0_, u1_, rv_)

    nc.compile()
    return nc


def _get_nc():
    if "nc" not in _CACHE:
        _CACHE["nc"] = _build()
    return _CACHE["nc"]


def kernel(x, wq, wk, wv, bq, bk, bv):
    x = np.asarray(x, dtype=np.float32)
    wq = np.asarray(wq, dtype=np.float32)
    wk = np.asarray(wk, dtype=np.float32)
    wv = np.asarray(wv, dtype=np.float32)
    bq = np.asarray(bq, dtype=np.float32)
    bk = np.asarray(bk, dtype=np.float32)
    bv = np.asarray(bv, dtype=np.float32)

    nc = _get_nc()
    shared = {
        "wq_t": np.ascontiguousarray(wq.T),
        "wk_t": np.ascontiguousarray(wk.T),
        "wv_t": np.ascontiguousarray(wv.T),
        "bq": np.ascontiguousarray(bq.reshape(C, 1)),
        "bk": np.ascontiguousarray(bk.reshape(C, 1)),
        "bv": np.ascontiguousarray(bv.reshape(1, C)),
    }
    in_maps = [
        {"x": np.ascontiguousarray(x[b].reshape(C, NPIX)), **shared} for b in range(B)
    ]
    res = run_bass_kernel_spmd(nc, in_maps, core_ids=list(range(B)))
    out = np.stack([res.results[b]["out"] for b in range(B)])
    return out.reshape(B, C, 64, 64)
